# revision 17
# baseline (speedup 1.0000x reference)
"""GATv2 (2-layer, PyG defaults) on 8 Trainium2 NeuronCores via Bass/Tile.

Sharding: nodes partitioned into 8 contiguous shards (2500 nodes/core); edges
assigned to the core owning their dst node, sorted by dst. Each core processes
windows of 128 dst nodes; per-edge source features are batch-gathered with
dma_gather; the x_r[dst] expansion and the segment-softmax numerator /
denominator sums are one-hot matmuls on the TensorEngine. Softmax is computed
without max-subtraction (logits are O(10), exp is safe in fp32; alpha is
mathematically identical).

Perf structure (v4):
- SWDGE gathers round-robin across 4 gpsimd queues: descriptor generation for
  consecutive windows runs concurrently on different Q7 core pairs.
- Software-pipelined emission: window w's scatter matmuls + tail are emitted
  after window w+1's matmul loop, so the PE FIFO never head-of-line blocks on
  the softmax chain and the HAM clock stays warm.
- Linear-layer biases are folded into the x_r side (per-shard, 2500 rows)
  instead of a ones-row on x (which cost a full extra 128-feature chunk on
  the replicated 20k-row transform).
- Per-half-window batched DVE ops; AllGather split into 3 chunks issued as
  soon as their windows finish so the collective overlaps phase-2 compute.
"""

import math
from dataclasses import dataclass

import ml_dtypes
import numpy as np

import concourse.bacc as bacc
import concourse.bass as bass
import concourse.mybir as mybir
import concourse.tile as tile
from concourse import library_config
from concourse.bass_utils import run_bass_kernel_spmd

BF16 = ml_dtypes.bfloat16
FP32 = np.float32
AF = mybir.ActivationFunctionType
NQ = 4                                      # SWDGE queues for gather round-robin
CHUNKS = [(0, 10, 1280), (10, 17, 896), (17, 20, 324)]  # AllGather (w0, w1, rows)


@dataclass
class Cfg:
    n_nodes: int = 20000
    n_feats: int = 256
    heads: int = 8
    dim_h: int = 64
    n_cls: int = 16
    neg_slope: float = 0.2
    n_cores: int = 8

    def __post_init__(self):
        self.hd = self.heads * self.dim_h
        assert self.n_nodes % self.n_cores == 0
        self.shard = self.n_nodes // self.n_cores
        self.n_win = math.ceil(self.shard / 128)
        self.full_w = self.shard // 128          # windows with all 128 dsts
        self.rem = self.shard - self.full_w * 128
        self.fc = math.ceil(self.n_feats / 128)
        self.f_pad = self.fc * 128
        self.h4 = self.hd // 128
        self.n_node_tiles = math.ceil(self.n_nodes / 128)
        self.n_pad = self.n_node_tiles * 128


def _prep_host(cfg: Cfg, x, edge_index, W_l1, b_l1, W_r1, b_r1, att1, bias1,
               W_l2, b_l2, W_r2, b_r2, att2, bias2):
    """CPU-side sharding: edge partitioning/sorting, one-hot matrices, gather
    indices, weight packing. Returns (in_maps, meta)."""
    N, S, NC = cfg.n_nodes, cfg.shard, cfg.n_cores
    HD, NCLS, H, D = cfg.hd, cfg.n_cls, cfg.heads, cfg.dim_h

    ei = np.asarray(edge_index).astype(np.int64)
    loop = np.arange(N, dtype=np.int64)
    src_all = np.concatenate([ei[0], loop])
    dst_all = np.concatenate([ei[1], loop])

    per_core = []
    for c in range(NC):
        sel = (dst_all // S) == c
        src_c, dst_c = src_all[sel], dst_all[sel]
        order = np.argsort(dst_c, kind="stable")
        src_c, dst_c = src_c[order], dst_c[order]
        dstl = dst_c - c * S
        wins = []
        for w in range(cfg.n_win):
            m = (dstl // 128) == w
            wins.append((src_c[m], dstl[m] - w * 128))
        per_core.append(wins)

    # unify per-window tile counts across cores (SPMD: same program everywhere)
    T = [max(1, *(math.ceil(len(per_core[c][w][0]) / 128) for c in range(NC)))
         for w in range(cfg.n_win)]
    toff = np.concatenate([[0], np.cumsum(T)]).astype(int)
    TOT = int(toff[-1])

    # layer-2 gather reads the chunk-ordered AllGather output
    ch_start = [0]
    for (_, _, r) in CHUNKS:
        ch_start.append(ch_start[-1] + r)
    assert ch_start[-1] == S

    def remap_l2(n):
        c, off = np.divmod(n, S)
        row = np.zeros_like(n)
        for k, (_, _, r) in enumerate(CHUNKS):
            sel = (off >= ch_start[k]) & (off < ch_start[k + 1])
            row = np.where(sel, NC * ch_start[k] + c * r + (off - ch_start[k]),
                           row)
        return row

    Ms, MTs, IDXs, IDX2s = [], [], [], []
    for c in range(NC):
        M = np.zeros((TOT, 128, 128), dtype=BF16)
        idx_flat = np.zeros((TOT * 128,), dtype=np.int64)
        for w in range(cfg.n_win):
            src_w, dloc = per_core[c][w]
            n = len(src_w)
            base = int(toff[w]) * 128
            idx_flat[base:base + n] = src_w
            ti = base + np.arange(n)
            M[ti // 128, ti % 128, dloc] = 1.0
        MT = np.ascontiguousarray(M.transpose(0, 2, 1))
        idx16 = idx_flat.astype(np.int16).reshape(-1, 16).T
        idx2_16 = remap_l2(idx_flat).astype(np.int16).reshape(-1, 16).T
        Ms.append(M)
        MTs.append(MT)
        IDXs.append(np.ascontiguousarray(np.tile(idx16, (8, 1))))
        IDX2s.append(np.ascontiguousarray(np.tile(idx2_16, (8, 1))))

    s = cfg.neg_slope
    a1 = np.asarray(att1, np.float64).reshape(HD)
    a2 = np.asarray(att2, np.float64).reshape(NCLS)
    Wl1f = np.asarray(W_l1, np.float64)
    Wr1f = np.asarray(W_r1, np.float64)
    # per-node attention dots (the decomposable s*z part of lrelu):
    # WA[f, h] = s * sum_d W[f, h*D+d] * att1[h, d]
    WAl = s * np.einsum("fhd,hd->fh", Wl1f.reshape(cfg.n_feats, H, D),
                        np.asarray(att1, np.float64))
    WAr = s * np.einsum("fhd,hd->fh", Wr1f.reshape(cfg.n_feats, H, D),
                        np.asarray(att1, np.float64))
    # both layer-1 biases ride the x_r side (z = xl + xr is all that matters)
    bsum1 = np.asarray(b_l1, np.float64) + np.asarray(b_r1, np.float64)
    xrb = np.concatenate([
        bsum1,
        s * np.einsum("hd,hd->h", np.asarray(att1, np.float64),
                      bsum1.reshape(H, D))])
    Wl2c = np.concatenate([np.asarray(W_l2, np.float64),
                           (s * np.asarray(W_l2, np.float64) @ a2)[:, None]], axis=1)
    Wr2c = np.concatenate([np.asarray(W_r2, np.float64),
                           (s * np.asarray(W_r2, np.float64) @ a2)[:, None]], axis=1)
    c0 = float(s * a2 @ (np.asarray(b_l2, np.float64) + np.asarray(b_r2, np.float64)))

    x = np.asarray(x, dtype=np.float32)
    xT = np.zeros((cfg.f_pad, cfg.n_pad), dtype=BF16)
    xT[:cfg.n_feats, :N] = x.T.astype(BF16)

    xTs = []
    for c in range(NC):
        xs = np.zeros((cfg.f_pad, cfg.n_win * 128), dtype=BF16)
        xs[:, :S] = xT[:, c * S:c * S + S]
        xTs.append(xs)

    rep = lambda v, dt: np.ascontiguousarray(
        np.tile(np.asarray(v, dtype=dt).reshape(1, -1), (128, 1)))

    common = dict(
        xT=xT,
        Wl1=Wl1f.astype(BF16),
        Wr1=Wr1f.astype(BF16),
        WAl=WAl.astype(BF16),
        WAr=WAr.astype(BF16),
        Wl2=Wl2c.astype(BF16),
        Wr2=Wr2c.astype(BF16),
        att1_rep=rep((1 - s) * a1, BF16),
        att2_rep=rep((1 - s) * a2, BF16),
        c0_rep=np.full((128, 1), c0, dtype=FP32),
        bias1_rep=rep(bias1, FP32),
        xrb_rep=rep(xrb, FP32),
        # out2 = sum_e alpha * xl2[src] with xl2 = h@W_l2 + b_l2; the gathered
        # x2g carries only h@W_l2, and sum(alpha) == 1, so fold b_l2 here.
        bias2_rep=rep(np.asarray(bias2, np.float64) + np.asarray(b_l2, np.float64),
                      FP32),
        b2sum_rep=rep(np.asarray(b_l2, np.float64) + np.asarray(b_r2, np.float64),
                      FP32),
        ident=np.eye(128, dtype=BF16),
    )
    in_maps = []
    for c in range(NC):
        m = dict(common)
        m["xTs"] = xTs[c]
        m["Mmat"] = Ms[c]
        m["MTmat"] = MTs[c]
        m["idx16"] = IDXs[c]
        m["idx16b"] = IDX2s[c]
        in_maps.append(m)
    meta = dict(T=T, toff=toff, TOT=TOT)
    return in_maps, meta


def build_program(cfg: Cfg, meta):
    T, toff, TOT = meta["T"], meta["toff"], meta["TOT"]
    Tmax = max(T)
    NW, S = cfg.n_win, cfg.shard
    HD, NCLS, H, D = cfg.hd, cfg.n_cls, cfg.heads, cfg.dim_h
    FC, H4 = cfg.fc, cfg.h4
    HDA = HD + H          # xl columns + per-node attention-dot columns
    EL1 = 640             # gather row width for layer 1 (>= HDA, bytes % 256 == 0)
    NC2 = NCLS + 1        # layer-2 projection + its attention-dot column
    dt = mybir.dt

    nc = bacc.Bacc("TRN2", target_bir_lowering=False, debug=False,
                   enable_asserts=True, num_devices=cfg.n_cores,
                   num_swdge_queues=NQ)

    ti = lambda n, s, d: nc.dram_tensor(n, s, d, kind="ExternalInput")
    xT_d = ti("xT", [cfg.f_pad, cfg.n_pad], dt.bfloat16)
    xTs_d = ti("xTs", [cfg.f_pad, NW * 128], dt.bfloat16)
    Wl1_d = ti("Wl1", [cfg.f_pad, HD], dt.bfloat16)
    Wr1_d = ti("Wr1", [cfg.f_pad, HD], dt.bfloat16)
    WAl_d = ti("WAl", [cfg.f_pad, H], dt.bfloat16)
    WAr_d = ti("WAr", [cfg.f_pad, H], dt.bfloat16)
    Wl2_d = ti("Wl2", [HD, NC2], dt.bfloat16)
    Wr2_d = ti("Wr2", [HD, NC2], dt.bfloat16)
    att1_d = ti("att1_rep", [128, HD], dt.bfloat16)
    att2_d = ti("att2_rep", [128, NCLS], dt.bfloat16)
    bias1_d = ti("bias1_rep", [128, HD], dt.float32)
    bias2_d = ti("bias2_rep", [128, NCLS], dt.float32)
    b2sum_d = ti("b2sum_rep", [128, NCLS], dt.float32)
    xrb_d = ti("xrb_rep", [128, HDA], dt.float32)
    c0_d = ti("c0_rep", [128, 1], dt.float32)
    ident_d = ti("ident", [128, 128], dt.bfloat16)
    Mmat_d = ti("Mmat", [TOT, 128, 128], dt.bfloat16)
    MTmat_d = ti("MTmat", [TOT, 128, 128], dt.bfloat16)
    idx_d = ti("idx16", [128, TOT * 8], dt.int16)
    idx2_d = ti("idx16b", [128, TOT * 8], dt.int16)
    out_d = nc.dram_tensor("out", [S, NCLS], dt.float32, kind="ExternalOutput")

    ch_start = [0]
    for (_, _, r) in CHUNKS:
        ch_start.append(ch_start[-1] + r)

    with tile.TileContext(nc) as tc:
        with (
            tc.tile_pool(name="const", bufs=1) as constp,
            tc.tile_pool(name="dram", bufs=1, space="DRAM") as dramp,
            tc.tile_pool(name="persist", bufs=1) as persist,
        ):
            nc.gpsimd.load_library(library_config.mlp)

            def load_const(dram, shape, dtype):
                t = constp.tile(shape, dtype, tag=f"c_{dram.name}",
                                name=f"c_{dram.name}")
                nc.sync.dma_start(t[:], dram.ap())
                return t

            def load_chunked(dram, nchunk, ncol, dtype):
                # [nchunk*128, ncol] DRAM -> [128, nchunk, ncol] SBUF
                t = constp.tile([128, nchunk, ncol], dtype, tag=f"c_{dram.name}",
                                name=f"c_{dram.name}")
                nc.sync.dma_start(
                    t[:], dram.ap().rearrange("(c r) k -> r c k", r=128))
                return t

            ident = load_const(ident_d, [128, 128], dt.bfloat16)
            att1 = load_const(att1_d, [128, HD], dt.bfloat16)
            att2 = load_const(att2_d, [128, NCLS], dt.bfloat16)
            bias1 = load_const(bias1_d, [128, HD], dt.float32)
            bias2 = load_const(bias2_d, [128, NCLS], dt.float32)
            b2sum = load_const(b2sum_d, [128, NCLS], dt.float32)
            xrb = load_const(xrb_d, [128, HDA], dt.float32)
            c0 = load_const(c0_d, [128, 1], dt.float32)
            idx16 = load_const(idx_d, [128, TOT * 8], dt.int16)
            Wl1 = load_chunked(Wl1_d, FC, HD, dt.bfloat16)
            Wr1 = load_chunked(Wr1_d, FC, HD, dt.bfloat16)
            WAl = load_chunked(WAl_d, FC, H, dt.bfloat16)
            WAr = load_chunked(WAr_d, FC, H, dt.bfloat16)
            Wl2 = load_chunked(Wl2_d, H4, NC2, dt.bfloat16)
            Wr2 = load_chunked(Wr2_d, H4, NC2, dt.bfloat16)

            xl_dram = dramp.tile([cfg.n_pad, EL1], dt.bfloat16)
            xl2_shard = dramp.tile([S, 128], dt.bfloat16)
            xl2_full = dramp.tile([cfg.n_nodes, 128], dt.bfloat16)
            xr_sb = persist.tile([128, NW, HDA], dt.bfloat16)
            hT_sb = persist.tile([128, H4, NW, 128], dt.bfloat16)
            xr2_sb = persist.tile([128, NW, NC2], dt.bfloat16)
            xl2_stage = persist.tile([128, NW, 128], dt.bfloat16)
            out_stage = persist.tile([128, NW, NCLS], dt.float32)

            # ---- phase 1: xl/al (full graph -> DRAM), xr/ar (own shard -> SBUF) ----
            with (
                tc.tile_pool(name="p1sb", bufs=3) as p1sb,
                tc.tile_pool(name="p1ps", bufs=3, space="PSUM") as p1ps,
                tc.tile_pool(name="p1ps2", bufs=3, space="PSUM") as p1ps2,
            ):
                nc.vector.memset(xl2_stage[:], 0.0)
                nt = 0
                while nt < cfg.n_node_tiles:
                    u = min(4, cfg.n_node_tiles - nt)
                    lt = p1sb.tile([128, FC, 4 * 128], dt.bfloat16, tag="lhsx")
                    nc.sync.dma_start(
                        lt[:, :, :u * 128],
                        xT_d.ap()[:, nt * 128:(nt + u) * 128]
                        .rearrange("(c r) k -> r c k", r=128))
                    ev = p1sb.tile([128, 4, HDA], dt.bfloat16, tag="ev")
                    for ui in range(u):
                        ps = p1ps.tile([128, HD], dt.float32, tag="p1")
                        psa = p1ps2.tile([128, H], dt.float32, tag="p1a")
                        for ci in range(FC):
                            nc.tensor.matmul(ps[:], lt[:, ci, ui * 128:(ui + 1) * 128],
                                             Wl1[:, ci, :],
                                             start=(ci == 0), stop=(ci == FC - 1))
                            nc.tensor.matmul(psa[:], lt[:, ci, ui * 128:(ui + 1) * 128],
                                             WAl[:, ci, :],
                                             start=(ci == 0), stop=(ci == FC - 1))
                        if ui % 2 == 0:
                            nc.scalar.activation(ev[:, ui, :HD], ps[:], AF.Copy)
                            nc.scalar.activation(ev[:, ui, HD:HDA], psa[:], AF.Copy)
                        else:
                            nc.vector.tensor_copy(ev[:, ui, :HD], ps[:])
                            nc.vector.tensor_copy(ev[:, ui, HD:HDA], psa[:])
                    nc.sync.dma_start(
                        xl_dram[nt * 128:(nt + u) * 128, :HDA]
                        .rearrange("(u p) k -> p u k", p=128),
                        ev[:, :u, :])
                    nt += u
                for w in range(NW):
                    lt = p1sb.tile([128, FC, 4 * 128], dt.bfloat16, tag="lhsx")
                    nc.sync.dma_start(
                        lt[:, :, :128],
                        xTs_d.ap()[:, w * 128:(w + 1) * 128]
                        .rearrange("(c r) k -> r c k", r=128))
                    ps = p1ps.tile([128, HD], dt.float32, tag="p1")
                    psa = p1ps2.tile([128, H], dt.float32, tag="p1a")
                    for ci in range(FC):
                        nc.tensor.matmul(ps[:], lt[:, ci, :128], Wr1[:, ci, :],
                                         start=(ci == 0), stop=(ci == FC - 1))
                        nc.tensor.matmul(psa[:], lt[:, ci, :128], WAr[:, ci, :],
                                         start=(ci == 0), stop=(ci == FC - 1))
                    # fold both layer-1 biases (and their attention dots) in here
                    nc.vector.tensor_add(xr_sb[:, w, :HD], ps[:], xrb[:, :HD])
                    nc.vector.tensor_add(xr_sb[:, w, HD:HDA], psa[:], xrb[:, HD:HDA])

            # ---- phase 2: layer-1 edges + projections + chunked AllGather ----
            with (
                tc.tile_pool(name="p2g", bufs=4) as p2g,
                tc.tile_pool(name="p2m", bufs=1) as p2m,
                tc.tile_pool(name="p2w", bufs=2) as p2wp,
                tc.tile_pool(name="p2ps", bufs=3, space="PSUM") as p2ps,
                tc.tile_pool(name="p2lg", bufs=1, space="PSUM") as p2lg,
                tc.tile_pool(name="p2acc", bufs=1, space="PSUM") as p2acc,
                tc.tile_pool(name="p2dacc", bufs=1, space="PSUM") as p2dacc,
                tc.tile_pool(name="p2hT", bufs=1, space="PSUM") as p2hT,
                tc.tile_pool(name="p2pj", bufs=1, space="PSUM") as p2pj,
                tc.tile_pool(name="p2s", bufs=2) as p2s,
            ):
                st = {}

                def p2_front(w):
                    Tw = T[w]
                    base = int(toff[w])
                    xlg = p2g.tile([128, Tmax, EL1], dt.bfloat16, tag="xlg")
                    nc.gpsimd.dma_gather(
                        xlg[:, :Tw, :], xl_dram[:],
                        idx16[:, base * 8:(base + Tw) * 8],
                        Tw * 128, Tw * 128, EL1, single_packet=False,
                        queue_num=w % NQ)
                    Mw = p2wp.tile([128, Tmax, 128], dt.bfloat16, tag="Mw")
                    MTw = p2wp.tile([128, Tmax, 128], dt.bfloat16, tag="MTw")
                    nc.sync.dma_start(Mw[:, :Tw, :],
                                      Mmat_d.ap()[base:base + Tw]
                                      .rearrange("t p k -> p t k"))
                    nc.sync.dma_start(MTw[:, :Tw, :],
                                      MTmat_d.ap()[base:base + Tw]
                                      .rearrange("t p k -> p t k"))
                    m_w = p2m.tile([128, Tmax, HD], dt.bfloat16, tag="m")
                    mp_lg = p2lg.tile([128, Tmax * H], dt.float32, tag="lgacc")
                    lgS = p2s.tile([128, Tmax * H], dt.float32, tag="lgS")
                    red = p2s.tile([128, Tmax * H], dt.float32, tag="red")
                    lg = p2s.tile([128, Tmax * H], dt.float32, tag="lg")
                    p_w = p2s.tile([128, Tmax * H], dt.bfloat16, tag="p")
                    h1 = (Tw + 1) // 2
                    for t in range(Tw):
                        mp = p2ps.tile([128, HD], dt.float32, tag="mpre")
                        nc.tensor.matmul(mp[:], MTw[:, t, :], xr_sb[:, w, :HD],
                                         start=True, stop=False)
                        nc.tensor.matmul(mp_lg[:, t * H:(t + 1) * H], MTw[:, t, :],
                                         xr_sb[:, w, HD:HDA], start=True, stop=False)
                        nc.tensor.matmul(mp[:], ident[:], xlg[:, t, :HD],
                                         start=False, stop=True)
                        nc.tensor.matmul(mp_lg[:, t * H:(t + 1) * H], ident[:],
                                         xlg[:, t, HD:HDA], start=False, stop=True)
                        nc.scalar.activation(m_w[:, t, :], mp[:], AF.Relu)
                        if t == h1 - 1 or t == Tw - 1:
                            lo = 0 if t == h1 - 1 else h1
                            hi = t + 1
                            sl = slice(lo * H, hi * H)
                            nc.scalar.activation(lgS[:, sl], mp_lg[:, sl], AF.Copy)
                            nc.vector.tensor_mul(
                                m_w[:, lo:hi, :], m_w[:, lo:hi, :],
                                att1[:].rearrange("p (o k) -> p o k", o=1)
                                    .to_broadcast([128, hi - lo, HD]))
                            nc.vector.tensor_reduce(
                                out=red[:, sl],
                                in_=m_w[:, lo:hi, :]
                                    .rearrange("p t (h d) -> p (t h) d", h=H),
                                op=mybir.AluOpType.add, axis=mybir.AxisListType.X)
                            nc.vector.tensor_add(lg[:, sl], red[:, sl], lgS[:, sl])
                            nc.scalar.activation(p_w[:, sl], lg[:, sl], AF.Exp)
                    # numerator p-muls last: they only feed next window's
                    # scatter, so keep them off the critical DVE chain.
                    # (per-tile 3-D broadcast keeps the DVE in 2x mode)
                    for tt in range(Tw):
                        nc.vector.tensor_mul(
                            xlg[:, tt, :HD]
                                .rearrange("p (h d) -> p h d", h=H),
                            xlg[:, tt, :HD]
                                .rearrange("p (h d) -> p h d", h=H),
                            p_w[:, tt * H:(tt + 1) * H]
                                .rearrange("p (h o) -> p h o", o=1)
                                .to_broadcast([128, H, D]))
                    st[w] = (xlg, Mw, p_w)

                def p2_back(w):
                    Tw = T[w]
                    xlg, Mw, p_w = st.pop(w)
                    out_ps = p2acc.tile([128, HD], dt.float32, tag="oacc")
                    den_ps = p2dacc.tile([128, H], dt.float32, tag="dacc")
                    for t in range(Tw):
                        nc.tensor.matmul(out_ps[:], Mw[:, t, :], xlg[:, t, :HD],
                                         start=(t == 0), stop=(t == Tw - 1))
                        nc.tensor.matmul(den_ps[:], Mw[:, t, :],
                                         p_w[:, t * H:(t + 1) * H],
                                         start=(t == 0), stop=(t == Tw - 1))
                    den = p2s.tile([128, H], dt.float32, tag="den", bufs=1)
                    rec = p2s.tile([128, H], dt.float32, tag="rec", bufs=1)
                    nc.vector.tensor_scalar_max(den[:], den_ps[:], 1e-30)
                    nc.vector.reciprocal(rec[:], den[:])
                    tmp = p2s.tile([128, HD], dt.float32, tag="tmp", bufs=1)
                    nc.vector.tensor_mul(
                        tmp[:].rearrange("p (h d) -> p h d", h=H),
                        out_ps[:].rearrange("p (h d) -> p h d", h=H),
                        rec[:].rearrange("p (h o) -> p h o", o=1)
                            .to_broadcast([128, H, D]))
                    nc.vector.tensor_add(tmp[:], tmp[:], bias1[:])
                    h_w = p2s.tile([128, HD], dt.bfloat16, tag="hw", bufs=1)
                    nc.scalar.activation(h_w[:], tmp[:], AF.Relu)
                    hT_ps = p2hT.tile([128, H4, 128], dt.bfloat16, tag="hT")
                    for c4 in range(H4):
                        nc.tensor.transpose(hT_ps[:, c4, :],
                                            h_w[:, c4 * 128:(c4 + 1) * 128], ident[:])
                    nc.scalar.activation(hT_sb[:, :, w, :], hT_ps[:], AF.Copy)
                    pj = p2pj.tile([128, NC2], dt.float32, tag="pj")
                    for c4 in range(H4):
                        nc.tensor.matmul(pj[:], hT_sb[:, c4, w, :], Wl2[:, c4, :],
                                         start=(c4 == 0), stop=(c4 == H4 - 1))
                    nc.scalar.activation(xl2_stage[:, w, :NC2], pj[:], AF.Copy)
                    pj = p2pj.tile([128, NC2], dt.float32, tag="pj")
                    for c4 in range(H4):
                        nc.tensor.matmul(pj[:], hT_sb[:, c4, w, :], Wr2[:, c4, :],
                                         start=(c4 == 0), stop=(c4 == H4 - 1))
                    nc.scalar.activation(xr2_sb[:, w, :], pj[:], AF.Copy)
                    for k, (w0, w1, rows) in enumerate(CHUNKS):
                        if w != w1 - 1:
                            continue
                        s0 = ch_start[k]
                        fw = rows // 128
                        nc.sync.dma_start(
                            xl2_shard[s0:s0 + fw * 128, :]
                            .rearrange("(w p) k -> p w k", p=128),
                            xl2_stage[:, w0:w0 + fw, :])
                        if rows % 128:
                            nc.sync.dma_start(
                                xl2_shard[s0 + fw * 128:s0 + rows, :],
                                xl2_stage[:rows % 128, w0 + fw, :])
                        nc.gpsimd.collective_compute(
                            "AllGather", mybir.AluOpType.bypass,
                            replica_groups=[list(range(cfg.n_cores))],
                            ins=[xl2_shard[s0:s0 + rows]],
                            outs=[xl2_full[cfg.n_cores * s0:
                                           cfg.n_cores * (s0 + rows)]])

                for w in range(NW):
                    p2_front(w)
                    if w > 0:
                        p2_back(w - 1)
                p2_back(NW - 1)

            # ---- phase 4: layer-2 edge processing ----
            with (
                tc.tile_pool(name="p4g", bufs=5) as p4g,
                tc.tile_pool(name="p4w", bufs=2) as p4wp,
                tc.tile_pool(name="p4c", bufs=1) as p4c,
                tc.tile_pool(name="p4ps", bufs=3, space="PSUM") as p4ps,
                tc.tile_pool(name="p4acc", bufs=2, space="PSUM") as p4acc,
                tc.tile_pool(name="p4s", bufs=2) as p4s,
            ):
                idx16b = p4c.tile([128, TOT * 8], dt.int16, tag="idx2")
                nc.sync.dma_start(idx16b[:], idx2_d.ap())
                st4 = {}

                def p4_front(w):
                    Tw = T[w]
                    base = int(toff[w])
                    x2g = p4g.tile([128, Tmax, 128], dt.bfloat16, tag="x2g")
                    nc.gpsimd.dma_gather(
                        x2g[:, :Tw, :], xl2_full[:],
                        idx16b[:, base * 8:(base + Tw) * 8],
                        Tw * 128, Tw * 128, 128, single_packet=False,
                        queue_num=w % NQ)
                    Mw = p4wp.tile([128, Tmax, 128], dt.bfloat16, tag="Mw2")
                    MTw = p4wp.tile([128, Tmax, 128], dt.bfloat16, tag="MTw2")
                    nc.sync.dma_start(Mw[:, :Tw, :],
                                      Mmat_d.ap()[base:base + Tw]
                                      .rearrange("t p k -> p t k"))
                    nc.sync.dma_start(MTw[:, :Tw, :],
                                      MTmat_d.ap()[base:base + Tw]
                                      .rearrange("t p k -> p t k"))
                    mp = p4ps.tile([128, Tmax, NC2], dt.float32, tag="m2")
                    m2b = p4s.tile([128, Tmax, NCLS], dt.float32, tag="m2b")
                    m2 = p4s.tile([128, Tmax, NCLS], dt.bfloat16, tag="m2l")
                    red2 = p4s.tile([128, Tmax], dt.float32, tag="red2")
                    lg2 = p4s.tile([128, Tmax], dt.float32, tag="lg2")
                    p2w_t = p4s.tile([128, Tmax], dt.bfloat16, tag="p2w")
                    h1 = (Tw + 1) // 2
                    for t in range(Tw):
                        nc.tensor.matmul(mp[:, t, :], MTw[:, t, :],
                                         xr2_sb[:, w, :], start=True, stop=False)
                        nc.tensor.matmul(mp[:, t, :], ident[:],
                                         x2g[:, t, :NC2], start=False, stop=True)
                        if t == h1 - 1 or t == Tw - 1:
                            lo = 0 if t == h1 - 1 else h1
                            hi = t + 1
                            n = hi - lo
                            nc.vector.tensor_add(
                                m2b[:, lo:hi, :], mp[:, lo:hi, :NCLS],
                                b2sum[:, :].rearrange("p (o c) -> p o c", o=1)
                                    .to_broadcast([128, n, NCLS]))
                            nc.scalar.activation(m2[:, lo:hi, :],
                                                 m2b[:, lo:hi, :], AF.Relu)
                            nc.vector.tensor_mul(
                                m2[:, lo:hi, :], m2[:, lo:hi, :],
                                att2[:, :].rearrange("p (o c) -> p o c", o=1)
                                    .to_broadcast([128, n, NCLS]))
                            nc.vector.tensor_reduce(
                                out=red2[:, lo:hi], in_=m2[:, lo:hi, :],
                                op=mybir.AluOpType.add, axis=mybir.AxisListType.X)
                            nc.vector.tensor_add(lg2[:, lo:hi], red2[:, lo:hi],
                                                 mp[:, lo:hi, NCLS])
                            nc.scalar.activation(p2w_t[:, lo:hi], lg2[:, lo:hi],
                                                 AF.Exp, bias=c0[:])
                            nc.vector.tensor_mul(
                                x2g[:, lo:hi, :NCLS],
                                x2g[:, lo:hi, :NCLS],
                                p2w_t[:, lo:hi].rearrange("p (t o) -> p t o", o=1)
                                    .to_broadcast([128, n, NCLS]))
                    st4[w] = (x2g, Mw, p2w_t)

                def p4_back(w):
                    Tw = T[w]
                    x2g, Mw, p2w_t = st4.pop(w)
                    out2_ps = p4acc.tile([128, NCLS], dt.float32, tag="o2acc")
                    den2_ps = p4acc.tile([128, 1], dt.float32, tag="d2acc")
                    for t in range(Tw):
                        nc.tensor.matmul(out2_ps[:], Mw[:, t, :], x2g[:, t, :NCLS],
                                         start=(t == 0), stop=(t == Tw - 1))
                        nc.tensor.matmul(den2_ps[:], Mw[:, t, :], p2w_t[:, t:t + 1],
                                         start=(t == 0), stop=(t == Tw - 1))
                    den2 = p4s.tile([128, 1], dt.float32, tag="den2")
                    rec2 = p4s.tile([128, 1], dt.float32, tag="rec2")
                    nc.vector.tensor_scalar_max(den2[:], den2_ps[:], 1e-30)
                    nc.vector.reciprocal(rec2[:], den2[:])
                    tmp2 = p4s.tile([128, NCLS], dt.float32, tag="tmp2")
                    nc.vector.tensor_mul(
                        tmp2[:].rearrange("p (o c) -> p o c", o=1),
                        out2_ps[:].rearrange("p (o c) -> p o c", o=1),
                        rec2[:].rearrange("p (c o) -> p c o", c=1)
                            .to_broadcast([128, 1, NCLS]))
                    nc.vector.tensor_add(out_stage[:, w, :], tmp2[:], bias2[:])

                for w in range(NW):
                    p4_front(w)
                    if w > 0:
                        p4_back(w - 1)
                p4_back(NW - 1)
                fw, rem = cfg.full_w, cfg.rem
                nc.sync.dma_start(
                    out_d.ap()[:fw * 128, :].rearrange("(w p) k -> p w k", p=128),
                    out_stage[:, :fw, :])
                if rem:
                    nc.sync.dma_start(out_d.ap()[fw * 128:, :],
                                      out_stage[:rem, fw, :])

    nc.compile()
    return nc


_last_result = None


def kernel(**inputs) -> np.ndarray:
    global _last_result
    import os
    cfg = Cfg()
    in_maps, meta = _prep_host(cfg, **inputs)
    nc = build_program(cfg, meta)
    kw = {}
    if os.environ.get("GAT_TRACE"):
        kw = dict(trace=True, tmpdir=os.environ.get("GAT_TRACE_DIR") or None)
    res = run_bass_kernel_spmd(nc, in_maps, core_ids=list(range(cfg.n_cores)), **kw)
    _last_result = res
    out = np.concatenate([res.results[c]["out"] for c in range(cfg.n_cores)], axis=0)
    return out.astype(np.float32)


# revision 20
# speedup vs baseline: 1.0187x; 1.0187x over previous
"""GATv2 (2-layer, PyG defaults) on 8 Trainium2 NeuronCores via Bass/Tile.

Sharding: nodes partitioned into 8 contiguous shards (2500 nodes/core); edges
assigned to the core owning their dst node, sorted by dst. Each core processes
windows of 128 dst nodes; per-edge source features are batch-gathered with
dma_gather; the x_r[dst] expansion and the segment-softmax numerator /
denominator sums are one-hot matmuls on the TensorEngine. Softmax is computed
without max-subtraction (logits are O(10), exp is safe in fp32; alpha is
mathematically identical).

Perf structure (v4):
- SWDGE gathers round-robin across 4 gpsimd queues: descriptor generation for
  consecutive windows runs concurrently on different Q7 core pairs.
- Software-pipelined emission: window w's scatter matmuls + tail are emitted
  after window w+1's matmul loop, so the PE FIFO never head-of-line blocks on
  the softmax chain and the HAM clock stays warm.
- Linear-layer biases are folded into the x_r side (per-shard, 2500 rows)
  instead of a ones-row on x (which cost a full extra 128-feature chunk on
  the replicated 20k-row transform).
- Per-half-window batched DVE ops; AllGather split into 3 chunks issued as
  soon as their windows finish so the collective overlaps phase-2 compute.
"""

import math
from dataclasses import dataclass

import ml_dtypes
import numpy as np

import concourse.bacc as bacc
import concourse.bass as bass
import concourse.mybir as mybir
import concourse.tile as tile
from concourse import library_config
from concourse.bass_utils import run_bass_kernel_spmd

BF16 = ml_dtypes.bfloat16
FP32 = np.float32
AF = mybir.ActivationFunctionType
NQ = 4                                      # SWDGE queues for gather round-robin
CHUNKS = [(0, 10, 1280), (10, 17, 896), (17, 20, 324)]  # AllGather (w0, w1, rows)


@dataclass
class Cfg:
    n_nodes: int = 20000
    n_feats: int = 256
    heads: int = 8
    dim_h: int = 64
    n_cls: int = 16
    neg_slope: float = 0.2
    n_cores: int = 8

    def __post_init__(self):
        self.hd = self.heads * self.dim_h
        assert self.n_nodes % self.n_cores == 0
        self.shard = self.n_nodes // self.n_cores
        self.n_win = math.ceil(self.shard / 128)
        self.full_w = self.shard // 128          # windows with all 128 dsts
        self.rem = self.shard - self.full_w * 128
        self.fc = math.ceil(self.n_feats / 128)
        self.f_pad = self.fc * 128
        self.h4 = self.hd // 128
        self.n_node_tiles = math.ceil(self.n_nodes / 128)
        self.n_pad = self.n_node_tiles * 128


def _prep_host(cfg: Cfg, x, edge_index, W_l1, b_l1, W_r1, b_r1, att1, bias1,
               W_l2, b_l2, W_r2, b_r2, att2, bias2):
    """CPU-side sharding: edge partitioning/sorting, one-hot matrices, gather
    indices, weight packing. Returns (in_maps, meta)."""
    N, S, NC = cfg.n_nodes, cfg.shard, cfg.n_cores
    HD, NCLS, H, D = cfg.hd, cfg.n_cls, cfg.heads, cfg.dim_h

    ei = np.asarray(edge_index).astype(np.int64)
    loop = np.arange(N, dtype=np.int64)
    src_all = np.concatenate([ei[0], loop])
    dst_all = np.concatenate([ei[1], loop])

    per_core = []
    for c in range(NC):
        sel = (dst_all // S) == c
        src_c, dst_c = src_all[sel], dst_all[sel]
        order = np.argsort(dst_c, kind="stable")
        src_c, dst_c = src_c[order], dst_c[order]
        dstl = dst_c - c * S
        wins = []
        for w in range(cfg.n_win):
            m = (dstl // 128) == w
            wins.append((src_c[m], dstl[m] - w * 128))
        per_core.append(wins)

    # unify per-window tile counts across cores (SPMD: same program everywhere)
    T = [max(1, *(math.ceil(len(per_core[c][w][0]) / 128) for c in range(NC)))
         for w in range(cfg.n_win)]
    toff = np.concatenate([[0], np.cumsum(T)]).astype(int)
    TOT = int(toff[-1])

    # layer-2 gather reads the chunk-ordered AllGather output
    ch_start = [0]
    for (_, _, r) in CHUNKS:
        ch_start.append(ch_start[-1] + r)
    assert ch_start[-1] == S

    def remap_l2(n):
        c, off = np.divmod(n, S)
        row = np.zeros_like(n)
        for k, (_, _, r) in enumerate(CHUNKS):
            sel = (off >= ch_start[k]) & (off < ch_start[k + 1])
            row = np.where(sel, NC * ch_start[k] + c * r + (off - ch_start[k]),
                           row)
        return row

    Ms, MTs, IDXs, IDX2s = [], [], [], []
    for c in range(NC):
        M = np.zeros((TOT, 128, 128), dtype=BF16)
        idx_flat = np.zeros((TOT * 128,), dtype=np.int64)
        for w in range(cfg.n_win):
            src_w, dloc = per_core[c][w]
            n = len(src_w)
            base = int(toff[w]) * 128
            idx_flat[base:base + n] = src_w
            ti = base + np.arange(n)
            M[ti // 128, ti % 128, dloc] = 1.0
        MT = np.ascontiguousarray(M.transpose(0, 2, 1))
        idx16 = idx_flat.astype(np.int16).reshape(-1, 16).T
        idx2_16 = remap_l2(idx_flat).astype(np.int16).reshape(-1, 16).T
        Ms.append(M)
        MTs.append(MT)
        IDXs.append(np.ascontiguousarray(np.tile(idx16, (8, 1))))
        IDX2s.append(np.ascontiguousarray(np.tile(idx2_16, (8, 1))))

    s = cfg.neg_slope
    a1 = np.asarray(att1, np.float64).reshape(HD)
    a2 = np.asarray(att2, np.float64).reshape(NCLS)
    Wl1f = np.asarray(W_l1, np.float64)
    Wr1f = np.asarray(W_r1, np.float64)
    # per-node attention dots (the decomposable s*z part of lrelu):
    # WA[f, h] = s * sum_d W[f, h*D+d] * att1[h, d]
    WAl = s * np.einsum("fhd,hd->fh", Wl1f.reshape(cfg.n_feats, H, D),
                        np.asarray(att1, np.float64))
    WAr = s * np.einsum("fhd,hd->fh", Wr1f.reshape(cfg.n_feats, H, D),
                        np.asarray(att1, np.float64))
    # both layer-1 biases ride the x_r side (z = xl + xr is all that matters)
    bsum1 = np.asarray(b_l1, np.float64) + np.asarray(b_r1, np.float64)
    xrb = np.concatenate([
        bsum1,
        s * np.einsum("hd,hd->h", np.asarray(att1, np.float64),
                      bsum1.reshape(H, D))])
    Wl2c = np.concatenate([np.asarray(W_l2, np.float64),
                           (s * np.asarray(W_l2, np.float64) @ a2)[:, None]], axis=1)
    Wr2c = np.concatenate([np.asarray(W_r2, np.float64),
                           (s * np.asarray(W_r2, np.float64) @ a2)[:, None]], axis=1)
    c0 = float(s * a2 @ (np.asarray(b_l2, np.float64) + np.asarray(b_r2, np.float64)))

    x = np.asarray(x, dtype=np.float32)
    xT = np.zeros((cfg.f_pad, cfg.n_pad), dtype=BF16)
    xT[:cfg.n_feats, :N] = x.T.astype(BF16)

    xTs = []
    for c in range(NC):
        xs = np.zeros((cfg.f_pad, cfg.n_win * 128), dtype=BF16)
        xs[:, :S] = xT[:, c * S:c * S + S]
        xTs.append(xs)

    rep = lambda v, dt: np.ascontiguousarray(
        np.tile(np.asarray(v, dtype=dt).reshape(1, -1), (128, 1)))

    common = dict(
        xT=xT,
        Wl1=Wl1f.astype(BF16),
        Wr1=Wr1f.astype(BF16),
        WAl=WAl.astype(BF16),
        WAr=WAr.astype(BF16),
        Wl2=Wl2c.astype(BF16),
        Wr2=Wr2c.astype(BF16),
        att1_rep=rep((1 - s) * a1, BF16),
        att2_rep=rep((1 - s) * a2, BF16),
        c0_rep=np.full((128, 1), c0, dtype=FP32),
        bias1_rep=rep(bias1, FP32),
        xrb_rep=rep(xrb, FP32),
        # out2 = sum_e alpha * xl2[src] with xl2 = h@W_l2 + b_l2; the gathered
        # x2g carries only h@W_l2, and sum(alpha) == 1, so fold b_l2 here.
        bias2_rep=rep(np.asarray(bias2, np.float64) + np.asarray(b_l2, np.float64),
                      FP32),
        b2sum_rep=rep(np.asarray(b_l2, np.float64) + np.asarray(b_r2, np.float64),
                      FP32),
        ident=np.eye(128, dtype=BF16),
    )
    in_maps = []
    for c in range(NC):
        m = dict(common)
        m["xTs"] = xTs[c]
        m["Mmat"] = Ms[c]
        m["MTmat"] = MTs[c]
        m["idx16"] = IDXs[c]
        m["idx16b"] = IDX2s[c]
        in_maps.append(m)
    meta = dict(T=T, toff=toff, TOT=TOT)
    return in_maps, meta


def build_program(cfg: Cfg, meta):
    T, toff, TOT = meta["T"], meta["toff"], meta["TOT"]
    Tmax = max(T)
    NW, S = cfg.n_win, cfg.shard
    HD, NCLS, H, D = cfg.hd, cfg.n_cls, cfg.heads, cfg.dim_h
    FC, H4 = cfg.fc, cfg.h4
    HDA = HD + H          # xl columns + per-node attention-dot columns
    EL1 = 640             # gather row width for layer 1 (>= HDA, bytes % 256 == 0)
    NC2 = NCLS + 1        # layer-2 projection + its attention-dot column
    dt = mybir.dt

    nc = bacc.Bacc("TRN2", target_bir_lowering=False, debug=False,
                   enable_asserts=True, num_devices=cfg.n_cores,
                   num_swdge_queues=NQ)

    ti = lambda n, s, d: nc.dram_tensor(n, s, d, kind="ExternalInput")
    xT_d = ti("xT", [cfg.f_pad, cfg.n_pad], dt.bfloat16)
    xTs_d = ti("xTs", [cfg.f_pad, NW * 128], dt.bfloat16)
    Wl1_d = ti("Wl1", [cfg.f_pad, HD], dt.bfloat16)
    Wr1_d = ti("Wr1", [cfg.f_pad, HD], dt.bfloat16)
    WAl_d = ti("WAl", [cfg.f_pad, H], dt.bfloat16)
    WAr_d = ti("WAr", [cfg.f_pad, H], dt.bfloat16)
    Wl2_d = ti("Wl2", [HD, NC2], dt.bfloat16)
    Wr2_d = ti("Wr2", [HD, NC2], dt.bfloat16)
    att1_d = ti("att1_rep", [128, HD], dt.bfloat16)
    att2_d = ti("att2_rep", [128, NCLS], dt.bfloat16)
    bias1_d = ti("bias1_rep", [128, HD], dt.float32)
    bias2_d = ti("bias2_rep", [128, NCLS], dt.float32)
    b2sum_d = ti("b2sum_rep", [128, NCLS], dt.float32)
    xrb_d = ti("xrb_rep", [128, HDA], dt.float32)
    c0_d = ti("c0_rep", [128, 1], dt.float32)
    ident_d = ti("ident", [128, 128], dt.bfloat16)
    Mmat_d = ti("Mmat", [TOT, 128, 128], dt.bfloat16)
    MTmat_d = ti("MTmat", [TOT, 128, 128], dt.bfloat16)
    idx_d = ti("idx16", [128, TOT * 8], dt.int16)
    idx2_d = ti("idx16b", [128, TOT * 8], dt.int16)
    out_d = nc.dram_tensor("out", [S, NCLS], dt.float32, kind="ExternalOutput")

    ch_start = [0]
    for (_, _, r) in CHUNKS:
        ch_start.append(ch_start[-1] + r)

    with tile.TileContext(nc) as tc:
        with (
            tc.tile_pool(name="const", bufs=1) as constp,
            tc.tile_pool(name="dram", bufs=1, space="DRAM") as dramp,
            tc.tile_pool(name="persist", bufs=1) as persist,
        ):
            nc.gpsimd.load_library(library_config.mlp)

            def load_const(dram, shape, dtype):
                t = constp.tile(shape, dtype, tag=f"c_{dram.name}",
                                name=f"c_{dram.name}")
                nc.sync.dma_start(t[:], dram.ap())
                return t

            def load_chunked(dram, nchunk, ncol, dtype):
                # [nchunk*128, ncol] DRAM -> [128, nchunk, ncol] SBUF
                t = constp.tile([128, nchunk, ncol], dtype, tag=f"c_{dram.name}",
                                name=f"c_{dram.name}")
                nc.sync.dma_start(
                    t[:], dram.ap().rearrange("(c r) k -> r c k", r=128))
                return t

            ident = load_const(ident_d, [128, 128], dt.bfloat16)
            att1 = load_const(att1_d, [128, HD], dt.bfloat16)
            att2 = load_const(att2_d, [128, NCLS], dt.bfloat16)
            bias1 = load_const(bias1_d, [128, HD], dt.float32)
            bias2 = load_const(bias2_d, [128, NCLS], dt.float32)
            b2sum = load_const(b2sum_d, [128, NCLS], dt.float32)
            xrb = load_const(xrb_d, [128, HDA], dt.float32)
            c0 = load_const(c0_d, [128, 1], dt.float32)
            idx16 = load_const(idx_d, [128, TOT * 8], dt.int16)
            Wl1 = load_chunked(Wl1_d, FC, HD, dt.bfloat16)
            Wr1 = load_chunked(Wr1_d, FC, HD, dt.bfloat16)
            WAl = load_chunked(WAl_d, FC, H, dt.bfloat16)
            WAr = load_chunked(WAr_d, FC, H, dt.bfloat16)
            Wl2 = load_chunked(Wl2_d, H4, NC2, dt.bfloat16)
            Wr2 = load_chunked(Wr2_d, H4, NC2, dt.bfloat16)

            xl_dram = dramp.tile([cfg.n_pad, EL1], dt.bfloat16)
            xl2_shard = dramp.tile([S, 128], dt.bfloat16)
            xl2_full = dramp.tile([cfg.n_nodes, 128], dt.bfloat16)
            xr_sb = persist.tile([128, NW, HDA], dt.bfloat16)
            hT_sb = persist.tile([128, H4, NW, 128], dt.bfloat16)
            xr2_sb = persist.tile([128, NW, NC2], dt.bfloat16)
            xl2_stage = persist.tile([128, NW, 128], dt.bfloat16)
            out_stage = persist.tile([128, NW, NCLS], dt.float32)

            # ---- phase 1: xl/al (full graph -> DRAM), xr/ar (own shard -> SBUF) ----
            with (
                tc.tile_pool(name="p1sb", bufs=3) as p1sb,
                tc.tile_pool(name="p1ps", bufs=3, space="PSUM") as p1ps,
                tc.tile_pool(name="p1ps2", bufs=3, space="PSUM") as p1ps2,
            ):
                nc.vector.memset(xl2_stage[:], 0.0)
                nt = 0
                while nt < cfg.n_node_tiles:
                    u = min(4, cfg.n_node_tiles - nt)
                    lt = p1sb.tile([128, FC, 4 * 128], dt.bfloat16, tag="lhsx")
                    nc.sync.dma_start(
                        lt[:, :, :u * 128],
                        xT_d.ap()[:, nt * 128:(nt + u) * 128]
                        .rearrange("(c r) k -> r c k", r=128))
                    ev = p1sb.tile([128, 4, HDA], dt.bfloat16, tag="ev")
                    for ui in range(u):
                        ps = p1ps.tile([128, HD], dt.float32, tag="p1")
                        psa = p1ps2.tile([128, H], dt.float32, tag="p1a")
                        for ci in range(FC):
                            nc.tensor.matmul(ps[:], lt[:, ci, ui * 128:(ui + 1) * 128],
                                             Wl1[:, ci, :],
                                             start=(ci == 0), stop=(ci == FC - 1))
                            nc.tensor.matmul(psa[:], lt[:, ci, ui * 128:(ui + 1) * 128],
                                             WAl[:, ci, :],
                                             start=(ci == 0), stop=(ci == FC - 1))
                        if ui % 2 == 0:
                            nc.scalar.activation(ev[:, ui, :HD], ps[:], AF.Copy)
                            nc.scalar.activation(ev[:, ui, HD:HDA], psa[:], AF.Copy)
                        else:
                            nc.vector.tensor_copy(ev[:, ui, :HD], ps[:])
                            nc.vector.tensor_copy(ev[:, ui, HD:HDA], psa[:])
                    nc.sync.dma_start(
                        xl_dram[nt * 128:(nt + u) * 128, :HDA]
                        .rearrange("(u p) k -> p u k", p=128),
                        ev[:, :u, :])
                    nt += u
                for w in range(NW):
                    lt = p1sb.tile([128, FC, 4 * 128], dt.bfloat16, tag="lhsx")
                    nc.sync.dma_start(
                        lt[:, :, :128],
                        xTs_d.ap()[:, w * 128:(w + 1) * 128]
                        .rearrange("(c r) k -> r c k", r=128))
                    ps = p1ps.tile([128, HD], dt.float32, tag="p1")
                    psa = p1ps2.tile([128, H], dt.float32, tag="p1a")
                    for ci in range(FC):
                        nc.tensor.matmul(ps[:], lt[:, ci, :128], Wr1[:, ci, :],
                                         start=(ci == 0), stop=(ci == FC - 1))
                        nc.tensor.matmul(psa[:], lt[:, ci, :128], WAr[:, ci, :],
                                         start=(ci == 0), stop=(ci == FC - 1))
                    # fold both layer-1 biases (and their attention dots) in here
                    nc.vector.tensor_add(xr_sb[:, w, :HD], ps[:], xrb[:, :HD])
                    nc.vector.tensor_add(xr_sb[:, w, HD:HDA], psa[:], xrb[:, HD:HDA])

            # ---- phase 2: layer-1 edges + projections + chunked AllGather ----
            with (
                tc.tile_pool(name="p2g", bufs=4) as p2g,
                tc.tile_pool(name="p2m", bufs=1) as p2m,
                tc.tile_pool(name="p2w", bufs=2) as p2wp,
                tc.tile_pool(name="p2ps", bufs=2, space="PSUM") as p2ps,
                tc.tile_pool(name="p2lg", bufs=1, space="PSUM") as p2lg,
                tc.tile_pool(name="p2acc", bufs=1, space="PSUM") as p2acc,
                tc.tile_pool(name="p2dacc", bufs=1, space="PSUM") as p2dacc,
                tc.tile_pool(name="p2hT", bufs=1, space="PSUM") as p2hT,
                tc.tile_pool(name="p2pj", bufs=1, space="PSUM") as p2pj,
                tc.tile_pool(name="p2s", bufs=2) as p2s,
            ):
                st = {}

                def p2_front(w):
                    Tw = T[w]
                    base = int(toff[w])
                    xlg = p2g.tile([128, Tmax, EL1], dt.bfloat16, tag="xlg")
                    nc.gpsimd.dma_gather(
                        xlg[:, :Tw, :], xl_dram[:],
                        idx16[:, base * 8:(base + Tw) * 8],
                        Tw * 128, Tw * 128, EL1, single_packet=False,
                        queue_num=w % NQ)
                    Mw = p2wp.tile([128, Tmax, 128], dt.bfloat16, tag="Mw")
                    MTw = p2wp.tile([128, Tmax, 128], dt.bfloat16, tag="MTw")
                    nc.sync.dma_start(Mw[:, :Tw, :],
                                      Mmat_d.ap()[base:base + Tw]
                                      .rearrange("t p k -> p t k"))
                    nc.sync.dma_start(MTw[:, :Tw, :],
                                      MTmat_d.ap()[base:base + Tw]
                                      .rearrange("t p k -> p t k"))
                    m_w = p2m.tile([128, Tmax, HD], dt.bfloat16, tag="m")
                    mp_lg = p2lg.tile([128, Tmax * H], dt.float32, tag="lgacc")
                    lgS = p2s.tile([128, Tmax * H], dt.float32, tag="lgS")
                    red = p2s.tile([128, Tmax * H], dt.float32, tag="red")
                    lg = p2s.tile([128, Tmax * H], dt.float32, tag="lg")
                    p_w = p2s.tile([128, Tmax * H], dt.bfloat16, tag="p")
                    h1 = (Tw + 1) // 2
                    for t in range(Tw):
                        mp = p2ps.tile([128, HD], dt.float32, tag="mpre")
                        nc.tensor.matmul(mp[:], MTw[:, t, :], xr_sb[:, w, :HD],
                                         start=True, stop=False)
                        nc.tensor.matmul(mp_lg[:, t * H:(t + 1) * H], MTw[:, t, :],
                                         xr_sb[:, w, HD:HDA], start=True, stop=False)
                        nc.tensor.matmul(mp[:], ident[:], xlg[:, t, :HD],
                                         start=False, stop=True)
                        nc.tensor.matmul(mp_lg[:, t * H:(t + 1) * H], ident[:],
                                         xlg[:, t, HD:HDA], start=False, stop=True)
                        nc.scalar.activation(m_w[:, t, :], mp[:], AF.Relu)
                        if t == h1 - 1 or t == Tw - 1:
                            lo = 0 if t == h1 - 1 else h1
                            hi = t + 1
                            sl = slice(lo * H, hi * H)
                            nc.scalar.activation(lgS[:, sl], mp_lg[:, sl], AF.Copy)
                            nc.vector.tensor_mul(
                                m_w[:, lo:hi, :], m_w[:, lo:hi, :],
                                att1[:].rearrange("p (o k) -> p o k", o=1)
                                    .to_broadcast([128, hi - lo, HD]))
                            nc.vector.tensor_reduce(
                                out=red[:, sl],
                                in_=m_w[:, lo:hi, :]
                                    .rearrange("p t (h d) -> p (t h) d", h=H),
                                op=mybir.AluOpType.add, axis=mybir.AxisListType.X)
                            nc.vector.tensor_add(lg[:, sl], red[:, sl], lgS[:, sl])
                            nc.scalar.activation(p_w[:, sl], lg[:, sl], AF.Exp)
                    # numerator p-muls last: they only feed next window's
                    # scatter, so keep them off the critical DVE chain.
                    # (per-tile 3-D broadcast keeps the DVE in 2x mode)
                    for tt in range(Tw):
                        nc.vector.tensor_mul(
                            xlg[:, tt, :HD]
                                .rearrange("p (h d) -> p h d", h=H),
                            xlg[:, tt, :HD]
                                .rearrange("p (h d) -> p h d", h=H),
                            p_w[:, tt * H:(tt + 1) * H]
                                .rearrange("p (h o) -> p h o", o=1)
                                .to_broadcast([128, H, D]))
                    st[w] = (xlg, Mw, p_w)

                st2 = {}

                def p2_backA(w):
                    Tw = T[w]
                    xlg, Mw, p_w = st.pop(w)
                    out_ps = p2acc.tile([128, HD], dt.float32, tag="oacc")
                    den_ps = p2dacc.tile([128, H], dt.float32, tag="dacc")
                    for t in range(Tw):
                        nc.tensor.matmul(out_ps[:], Mw[:, t, :], xlg[:, t, :HD],
                                         start=(t == 0), stop=(t == Tw - 1))
                        nc.tensor.matmul(den_ps[:], Mw[:, t, :],
                                         p_w[:, t * H:(t + 1) * H],
                                         start=(t == 0), stop=(t == Tw - 1))
                    den = p2s.tile([128, H], dt.float32, tag="den", bufs=1)
                    rec = p2s.tile([128, H], dt.float32, tag="rec", bufs=1)
                    nc.vector.tensor_scalar_max(den[:], den_ps[:], 1e-30)
                    nc.vector.reciprocal(rec[:], den[:])
                    tmp = p2s.tile([128, HD], dt.float32, tag="tmp", bufs=1)
                    nc.vector.tensor_mul(
                        tmp[:].rearrange("p (h d) -> p h d", h=H),
                        out_ps[:].rearrange("p (h d) -> p h d", h=H),
                        rec[:].rearrange("p (h o) -> p h o", o=1)
                            .to_broadcast([128, H, D]))
                    nc.vector.tensor_add(tmp[:], tmp[:], bias1[:])
                    h_w = p2s.tile([128, HD], dt.bfloat16, tag="hw", bufs=2)
                    nc.scalar.activation(h_w[:], tmp[:], AF.Relu)
                    st2[w] = h_w

                def p2_backB(w):
                    h_w = st2.pop(w)
                    hT_ps = p2hT.tile([128, H4, 128], dt.bfloat16, tag="hT")
                    for c4 in range(H4):
                        nc.tensor.transpose(hT_ps[:, c4, :],
                                            h_w[:, c4 * 128:(c4 + 1) * 128], ident[:])
                    nc.scalar.activation(hT_sb[:, :, w, :], hT_ps[:], AF.Copy)
                    pj = p2pj.tile([128, NC2], dt.float32, tag="pj")
                    for c4 in range(H4):
                        nc.tensor.matmul(pj[:], hT_sb[:, c4, w, :], Wl2[:, c4, :],
                                         start=(c4 == 0), stop=(c4 == H4 - 1))
                    nc.scalar.activation(xl2_stage[:, w, :NC2], pj[:], AF.Copy)
                    pj = p2pj.tile([128, NC2], dt.float32, tag="pj")
                    for c4 in range(H4):
                        nc.tensor.matmul(pj[:], hT_sb[:, c4, w, :], Wr2[:, c4, :],
                                         start=(c4 == 0), stop=(c4 == H4 - 1))
                    nc.scalar.activation(xr2_sb[:, w, :], pj[:], AF.Copy)
                    for k, (w0, w1, rows) in enumerate(CHUNKS):
                        if w != w1 - 1:
                            continue
                        s0 = ch_start[k]
                        fw = rows // 128
                        nc.sync.dma_start(
                            xl2_shard[s0:s0 + fw * 128, :]
                            .rearrange("(w p) k -> p w k", p=128),
                            xl2_stage[:, w0:w0 + fw, :])
                        if rows % 128:
                            nc.sync.dma_start(
                                xl2_shard[s0 + fw * 128:s0 + rows, :],
                                xl2_stage[:rows % 128, w0 + fw, :])
                        nc.gpsimd.collective_compute(
                            "AllGather", mybir.AluOpType.bypass,
                            replica_groups=[list(range(cfg.n_cores))],
                            ins=[xl2_shard[s0:s0 + rows]],
                            outs=[xl2_full[cfg.n_cores * s0:
                                           cfg.n_cores * (s0 + rows)]])

                for w in range(NW):
                    p2_front(w)
                    if w >= 1:
                        p2_backA(w - 1)
                    if w >= 2:
                        p2_backB(w - 2)
                p2_backA(NW - 1)
                p2_backB(NW - 2)
                p2_backB(NW - 1)

            # ---- phase 4: layer-2 edge processing ----
            with (
                tc.tile_pool(name="p4g", bufs=5) as p4g,
                tc.tile_pool(name="p4w", bufs=2) as p4wp,
                tc.tile_pool(name="p4c", bufs=1) as p4c,
                tc.tile_pool(name="p4ps", bufs=3, space="PSUM") as p4ps,
                tc.tile_pool(name="p4acc", bufs=2, space="PSUM") as p4acc,
                tc.tile_pool(name="p4s", bufs=2) as p4s,
            ):
                idx16b = p4c.tile([128, TOT * 8], dt.int16, tag="idx2")
                nc.sync.dma_start(idx16b[:], idx2_d.ap())
                st4 = {}

                def p4_front(w):
                    Tw = T[w]
                    base = int(toff[w])
                    x2g = p4g.tile([128, Tmax, 128], dt.bfloat16, tag="x2g")
                    nc.gpsimd.dma_gather(
                        x2g[:, :Tw, :], xl2_full[:],
                        idx16b[:, base * 8:(base + Tw) * 8],
                        Tw * 128, Tw * 128, 128, single_packet=False,
                        queue_num=w % NQ)
                    Mw = p4wp.tile([128, Tmax, 128], dt.bfloat16, tag="Mw2")
                    MTw = p4wp.tile([128, Tmax, 128], dt.bfloat16, tag="MTw2")
                    nc.sync.dma_start(Mw[:, :Tw, :],
                                      Mmat_d.ap()[base:base + Tw]
                                      .rearrange("t p k -> p t k"))
                    nc.sync.dma_start(MTw[:, :Tw, :],
                                      MTmat_d.ap()[base:base + Tw]
                                      .rearrange("t p k -> p t k"))
                    mp = p4ps.tile([128, Tmax, NC2], dt.float32, tag="m2")
                    m2b = p4s.tile([128, Tmax, NCLS], dt.float32, tag="m2b")
                    m2 = p4s.tile([128, Tmax, NCLS], dt.bfloat16, tag="m2l")
                    red2 = p4s.tile([128, Tmax], dt.float32, tag="red2")
                    lg2 = p4s.tile([128, Tmax], dt.float32, tag="lg2")
                    p2w_t = p4s.tile([128, Tmax], dt.bfloat16, tag="p2w")
                    h1 = (Tw + 1) // 2
                    for t in range(Tw):
                        nc.tensor.matmul(mp[:, t, :], MTw[:, t, :],
                                         xr2_sb[:, w, :], start=True, stop=False)
                        nc.tensor.matmul(mp[:, t, :], ident[:],
                                         x2g[:, t, :NC2], start=False, stop=True)
                        if t == h1 - 1 or t == Tw - 1:
                            lo = 0 if t == h1 - 1 else h1
                            hi = t + 1
                            n = hi - lo
                            nc.vector.tensor_add(
                                m2b[:, lo:hi, :], mp[:, lo:hi, :NCLS],
                                b2sum[:, :].rearrange("p (o c) -> p o c", o=1)
                                    .to_broadcast([128, n, NCLS]))
                            nc.scalar.activation(m2[:, lo:hi, :],
                                                 m2b[:, lo:hi, :], AF.Relu)
                            nc.vector.tensor_mul(
                                m2[:, lo:hi, :], m2[:, lo:hi, :],
                                att2[:, :].rearrange("p (o c) -> p o c", o=1)
                                    .to_broadcast([128, n, NCLS]))
                            nc.vector.tensor_reduce(
                                out=red2[:, lo:hi], in_=m2[:, lo:hi, :],
                                op=mybir.AluOpType.add, axis=mybir.AxisListType.X)
                            nc.vector.tensor_add(lg2[:, lo:hi], red2[:, lo:hi],
                                                 mp[:, lo:hi, NCLS])
                            nc.scalar.activation(p2w_t[:, lo:hi], lg2[:, lo:hi],
                                                 AF.Exp, bias=c0[:])
                            nc.vector.tensor_mul(
                                x2g[:, lo:hi, :NCLS],
                                x2g[:, lo:hi, :NCLS],
                                p2w_t[:, lo:hi].rearrange("p (t o) -> p t o", o=1)
                                    .to_broadcast([128, n, NCLS]))
                    st4[w] = (x2g, Mw, p2w_t)

                def p4_back(w):
                    Tw = T[w]
                    x2g, Mw, p2w_t = st4.pop(w)
                    out2_ps = p4acc.tile([128, NCLS], dt.float32, tag="o2acc")
                    den2_ps = p4acc.tile([128, 1], dt.float32, tag="d2acc")
                    for t in range(Tw):
                        nc.tensor.matmul(out2_ps[:], Mw[:, t, :], x2g[:, t, :NCLS],
                                         start=(t == 0), stop=(t == Tw - 1))
                        nc.tensor.matmul(den2_ps[:], Mw[:, t, :], p2w_t[:, t:t + 1],
                                         start=(t == 0), stop=(t == Tw - 1))
                    den2 = p4s.tile([128, 1], dt.float32, tag="den2")
                    rec2 = p4s.tile([128, 1], dt.float32, tag="rec2")
                    nc.vector.tensor_scalar_max(den2[:], den2_ps[:], 1e-30)
                    nc.vector.reciprocal(rec2[:], den2[:])
                    tmp2 = p4s.tile([128, NCLS], dt.float32, tag="tmp2")
                    nc.vector.tensor_mul(
                        tmp2[:].rearrange("p (o c) -> p o c", o=1),
                        out2_ps[:].rearrange("p (o c) -> p o c", o=1),
                        rec2[:].rearrange("p (c o) -> p c o", c=1)
                            .to_broadcast([128, 1, NCLS]))
                    nc.vector.tensor_add(out_stage[:, w, :], tmp2[:], bias2[:])

                for w in range(NW):
                    p4_front(w)
                    if w > 0:
                        p4_back(w - 1)
                p4_back(NW - 1)
                fw, rem = cfg.full_w, cfg.rem
                nc.sync.dma_start(
                    out_d.ap()[:fw * 128, :].rearrange("(w p) k -> p w k", p=128),
                    out_stage[:, :fw, :])
                if rem:
                    nc.sync.dma_start(out_d.ap()[fw * 128:, :],
                                      out_stage[:rem, fw, :])

    nc.compile()
    return nc


_last_result = None


def kernel(**inputs) -> np.ndarray:
    global _last_result
    import os
    cfg = Cfg()
    in_maps, meta = _prep_host(cfg, **inputs)
    nc = build_program(cfg, meta)
    kw = {}
    if os.environ.get("GAT_TRACE"):
        kw = dict(trace=True, tmpdir=os.environ.get("GAT_TRACE_DIR") or None)
    res = run_bass_kernel_spmd(nc, in_maps, core_ids=list(range(cfg.n_cores)), **kw)
    _last_result = res
    out = np.concatenate([res.results[c]["out"] for c in range(cfg.n_cores)], axis=0)
    return out.astype(np.float32)


# revision 21
# speedup vs baseline: 1.0409x; 1.0218x over previous
"""GATv2 (2-layer, PyG defaults) on 8 Trainium2 NeuronCores via Bass/Tile.

Sharding: nodes partitioned into 8 contiguous shards (2500 nodes/core); edges
assigned to the core owning their dst node, sorted by dst. Each core processes
windows of 128 dst nodes; per-edge source features are batch-gathered with
dma_gather; the x_r[dst] expansion and the segment-softmax numerator /
denominator sums are one-hot matmuls on the TensorEngine. Softmax is computed
without max-subtraction (logits are O(10), exp is safe in fp32; alpha is
mathematically identical).

Perf structure (v4):
- SWDGE gathers round-robin across 4 gpsimd queues: descriptor generation for
  consecutive windows runs concurrently on different Q7 core pairs.
- Software-pipelined emission: window w's scatter matmuls + tail are emitted
  after window w+1's matmul loop, so the PE FIFO never head-of-line blocks on
  the softmax chain and the HAM clock stays warm.
- Linear-layer biases are folded into the x_r side (per-shard, 2500 rows)
  instead of a ones-row on x (which cost a full extra 128-feature chunk on
  the replicated 20k-row transform).
- Per-half-window batched DVE ops; AllGather split into 3 chunks issued as
  soon as their windows finish so the collective overlaps phase-2 compute.
"""

import math
from dataclasses import dataclass

import ml_dtypes
import numpy as np

import concourse.bacc as bacc
import concourse.bass as bass
import concourse.mybir as mybir
import concourse.tile as tile
from concourse import library_config
from concourse.bass_utils import run_bass_kernel_spmd

BF16 = ml_dtypes.bfloat16
FP32 = np.float32
AF = mybir.ActivationFunctionType
NQ = 4                                      # SWDGE queues for gather round-robin
CHUNKS = [(0, 10, 1280), (10, 17, 896), (17, 20, 324)]  # AllGather (w0, w1, rows)


@dataclass
class Cfg:
    n_nodes: int = 20000
    n_feats: int = 256
    heads: int = 8
    dim_h: int = 64
    n_cls: int = 16
    neg_slope: float = 0.2
    n_cores: int = 8

    def __post_init__(self):
        self.hd = self.heads * self.dim_h
        assert self.n_nodes % self.n_cores == 0
        self.shard = self.n_nodes // self.n_cores
        self.n_win = math.ceil(self.shard / 128)
        self.full_w = self.shard // 128          # windows with all 128 dsts
        self.rem = self.shard - self.full_w * 128
        self.fc = math.ceil(self.n_feats / 128)
        self.f_pad = self.fc * 128
        self.h4 = self.hd // 128
        self.n_node_tiles = math.ceil(self.n_nodes / 128)
        self.n_pad = self.n_node_tiles * 128


def _prep_host(cfg: Cfg, x, edge_index, W_l1, b_l1, W_r1, b_r1, att1, bias1,
               W_l2, b_l2, W_r2, b_r2, att2, bias2):
    """CPU-side sharding: edge partitioning/sorting, one-hot matrices, gather
    indices, weight packing. Returns (in_maps, meta)."""
    N, S, NC = cfg.n_nodes, cfg.shard, cfg.n_cores
    HD, NCLS, H, D = cfg.hd, cfg.n_cls, cfg.heads, cfg.dim_h

    ei = np.asarray(edge_index).astype(np.int64)
    loop = np.arange(N, dtype=np.int64)
    src_all = np.concatenate([ei[0], loop])
    dst_all = np.concatenate([ei[1], loop])

    per_core = []
    for c in range(NC):
        sel = (dst_all // S) == c
        src_c, dst_c = src_all[sel], dst_all[sel]
        order = np.argsort(dst_c, kind="stable")
        src_c, dst_c = src_c[order], dst_c[order]
        dstl = dst_c - c * S
        wins = []
        for w in range(cfg.n_win):
            m = (dstl // 128) == w
            wins.append((src_c[m], dstl[m] - w * 128))
        per_core.append(wins)

    # unify per-window tile counts across cores (SPMD: same program everywhere)
    T = [max(1, *(math.ceil(len(per_core[c][w][0]) / 128) for c in range(NC)))
         for w in range(cfg.n_win)]
    toff = np.concatenate([[0], np.cumsum(T)]).astype(int)
    TOT = int(toff[-1])

    # layer-2 gather reads the chunk-ordered AllGather output
    ch_start = [0]
    for (_, _, r) in CHUNKS:
        ch_start.append(ch_start[-1] + r)
    assert ch_start[-1] == S

    def remap_l2(n):
        c, off = np.divmod(n, S)
        row = np.zeros_like(n)
        for k, (_, _, r) in enumerate(CHUNKS):
            sel = (off >= ch_start[k]) & (off < ch_start[k + 1])
            row = np.where(sel, NC * ch_start[k] + c * r + (off - ch_start[k]),
                           row)
        return row

    Ms, MTs, IDXs, IDX2s = [], [], [], []
    for c in range(NC):
        M = np.zeros((TOT, 128, 128), dtype=BF16)
        idx_flat = np.zeros((TOT * 128,), dtype=np.int64)
        for w in range(cfg.n_win):
            src_w, dloc = per_core[c][w]
            n = len(src_w)
            base = int(toff[w]) * 128
            idx_flat[base:base + n] = src_w
            ti = base + np.arange(n)
            M[ti // 128, ti % 128, dloc] = 1.0
        MT = np.ascontiguousarray(M.transpose(0, 2, 1))
        idx16 = idx_flat.astype(np.int16).reshape(-1, 16).T
        idx2_16 = remap_l2(idx_flat).astype(np.int16).reshape(-1, 16).T
        Ms.append(M)
        MTs.append(MT)
        IDXs.append(np.ascontiguousarray(np.tile(idx16, (8, 1))))
        IDX2s.append(np.ascontiguousarray(np.tile(idx2_16, (8, 1))))

    s = cfg.neg_slope
    a1 = np.asarray(att1, np.float64).reshape(HD)
    a2 = np.asarray(att2, np.float64).reshape(NCLS)
    Wl1f = np.asarray(W_l1, np.float64)
    Wr1f = np.asarray(W_r1, np.float64)
    # per-node attention dots (the decomposable s*z part of lrelu):
    # WA[f, h] = s * sum_d W[f, h*D+d] * att1[h, d]
    WAl = s * np.einsum("fhd,hd->fh", Wl1f.reshape(cfg.n_feats, H, D),
                        np.asarray(att1, np.float64))
    WAr = s * np.einsum("fhd,hd->fh", Wr1f.reshape(cfg.n_feats, H, D),
                        np.asarray(att1, np.float64))
    # both layer-1 biases ride the x_r side (z = xl + xr is all that matters)
    bsum1 = np.asarray(b_l1, np.float64) + np.asarray(b_r1, np.float64)
    xrb = np.concatenate([
        bsum1,
        s * np.einsum("hd,hd->h", np.asarray(att1, np.float64),
                      bsum1.reshape(H, D))])
    Wl2c = np.concatenate([np.asarray(W_l2, np.float64),
                           (s * np.asarray(W_l2, np.float64) @ a2)[:, None]], axis=1)
    Wr2c = np.concatenate([np.asarray(W_r2, np.float64),
                           (s * np.asarray(W_r2, np.float64) @ a2)[:, None]], axis=1)
    c0 = float(s * a2 @ (np.asarray(b_l2, np.float64) + np.asarray(b_r2, np.float64)))

    x = np.asarray(x, dtype=np.float32)
    xT = np.zeros((cfg.f_pad, cfg.n_pad), dtype=BF16)
    xT[:cfg.n_feats, :N] = x.T.astype(BF16)

    xTs = []
    for c in range(NC):
        xs = np.zeros((cfg.f_pad, cfg.n_win * 128), dtype=BF16)
        xs[:, :S] = xT[:, c * S:c * S + S]
        xTs.append(xs)

    rep = lambda v, dt: np.ascontiguousarray(
        np.tile(np.asarray(v, dtype=dt).reshape(1, -1), (128, 1)))

    common = dict(
        xT=xT,
        Wl1=Wl1f.astype(BF16),
        Wr1=Wr1f.astype(BF16),
        WAl=WAl.astype(BF16),
        WAr=WAr.astype(BF16),
        Wl2=Wl2c.astype(BF16),
        Wr2=Wr2c.astype(BF16),
        att1_rep=rep((1 - s) * a1, BF16),
        att2_rep=rep((1 - s) * a2, BF16),
        c0_rep=np.full((128, 1), c0, dtype=FP32),
        bias1_rep=rep(bias1, FP32),
        xrb_rep=rep(xrb, FP32),
        # out2 = sum_e alpha * xl2[src] with xl2 = h@W_l2 + b_l2; the gathered
        # x2g carries only h@W_l2, and sum(alpha) == 1, so fold b_l2 here.
        bias2_rep=rep(np.asarray(bias2, np.float64) + np.asarray(b_l2, np.float64),
                      FP32),
        b2sum_rep=rep(np.asarray(b_l2, np.float64) + np.asarray(b_r2, np.float64),
                      FP32),
        ident=np.eye(128, dtype=BF16),
    )
    in_maps = []
    for c in range(NC):
        m = dict(common)
        m["xTs"] = xTs[c]
        m["Mmat"] = Ms[c]
        m["MTmat"] = MTs[c]
        m["idx16"] = IDXs[c]
        m["idx16b"] = IDX2s[c]
        in_maps.append(m)
    meta = dict(T=T, toff=toff, TOT=TOT)
    return in_maps, meta


def build_program(cfg: Cfg, meta):
    T, toff, TOT = meta["T"], meta["toff"], meta["TOT"]
    Tmax = max(T)
    NW, S = cfg.n_win, cfg.shard
    HD, NCLS, H, D = cfg.hd, cfg.n_cls, cfg.heads, cfg.dim_h
    FC, H4 = cfg.fc, cfg.h4
    HDA = HD + H          # xl columns + per-node attention-dot columns
    EL1 = 640             # gather row width for layer 1 (>= HDA, bytes % 256 == 0)
    NC2 = NCLS + 1        # layer-2 projection + its attention-dot column
    dt = mybir.dt

    nc = bacc.Bacc("TRN2", target_bir_lowering=False, debug=False,
                   enable_asserts=True, num_devices=cfg.n_cores,
                   num_swdge_queues=NQ)

    ti = lambda n, s, d: nc.dram_tensor(n, s, d, kind="ExternalInput")
    xT_d = ti("xT", [cfg.f_pad, cfg.n_pad], dt.bfloat16)
    xTs_d = ti("xTs", [cfg.f_pad, NW * 128], dt.bfloat16)
    Wl1_d = ti("Wl1", [cfg.f_pad, HD], dt.bfloat16)
    Wr1_d = ti("Wr1", [cfg.f_pad, HD], dt.bfloat16)
    WAl_d = ti("WAl", [cfg.f_pad, H], dt.bfloat16)
    WAr_d = ti("WAr", [cfg.f_pad, H], dt.bfloat16)
    Wl2_d = ti("Wl2", [HD, NC2], dt.bfloat16)
    Wr2_d = ti("Wr2", [HD, NC2], dt.bfloat16)
    att1_d = ti("att1_rep", [128, HD], dt.bfloat16)
    att2_d = ti("att2_rep", [128, NCLS], dt.bfloat16)
    bias1_d = ti("bias1_rep", [128, HD], dt.float32)
    bias2_d = ti("bias2_rep", [128, NCLS], dt.float32)
    b2sum_d = ti("b2sum_rep", [128, NCLS], dt.float32)
    xrb_d = ti("xrb_rep", [128, HDA], dt.float32)
    c0_d = ti("c0_rep", [128, 1], dt.float32)
    ident_d = ti("ident", [128, 128], dt.bfloat16)
    Mmat_d = ti("Mmat", [TOT, 128, 128], dt.bfloat16)
    MTmat_d = ti("MTmat", [TOT, 128, 128], dt.bfloat16)
    idx_d = ti("idx16", [128, TOT * 8], dt.int16)
    idx2_d = ti("idx16b", [128, TOT * 8], dt.int16)
    out_d = nc.dram_tensor("out", [S, NCLS], dt.float32, kind="ExternalOutput")

    ch_start = [0]
    for (_, _, r) in CHUNKS:
        ch_start.append(ch_start[-1] + r)

    with tile.TileContext(nc) as tc:
        with (
            tc.tile_pool(name="const", bufs=1) as constp,
            tc.tile_pool(name="dram", bufs=1, space="DRAM") as dramp,
            tc.tile_pool(name="persist", bufs=1) as persist,
        ):
            nc.gpsimd.load_library(library_config.mlp)

            def load_const(dram, shape, dtype):
                t = constp.tile(shape, dtype, tag=f"c_{dram.name}",
                                name=f"c_{dram.name}")
                nc.sync.dma_start(t[:], dram.ap())
                return t

            def load_chunked(dram, nchunk, ncol, dtype):
                # [nchunk*128, ncol] DRAM -> [128, nchunk, ncol] SBUF
                t = constp.tile([128, nchunk, ncol], dtype, tag=f"c_{dram.name}",
                                name=f"c_{dram.name}")
                nc.sync.dma_start(
                    t[:], dram.ap().rearrange("(c r) k -> r c k", r=128))
                return t

            ident = load_const(ident_d, [128, 128], dt.bfloat16)
            att1 = load_const(att1_d, [128, HD], dt.bfloat16)
            att2 = load_const(att2_d, [128, NCLS], dt.bfloat16)
            bias1 = load_const(bias1_d, [128, HD], dt.float32)
            bias2 = load_const(bias2_d, [128, NCLS], dt.float32)
            b2sum = load_const(b2sum_d, [128, NCLS], dt.float32)
            xrb = load_const(xrb_d, [128, HDA], dt.float32)
            c0 = load_const(c0_d, [128, 1], dt.float32)
            idx16 = load_const(idx_d, [128, TOT * 8], dt.int16)
            Wl1 = load_chunked(Wl1_d, FC, HD, dt.bfloat16)
            Wr1 = load_chunked(Wr1_d, FC, HD, dt.bfloat16)
            WAl = load_chunked(WAl_d, FC, H, dt.bfloat16)
            WAr = load_chunked(WAr_d, FC, H, dt.bfloat16)
            Wl2 = load_chunked(Wl2_d, H4, NC2, dt.bfloat16)
            Wr2 = load_chunked(Wr2_d, H4, NC2, dt.bfloat16)

            xl_dram = dramp.tile([cfg.n_pad, EL1], dt.bfloat16)
            xl2_shard = dramp.tile([S, 128], dt.bfloat16)
            xl2_full = dramp.tile([cfg.n_nodes, 128], dt.bfloat16)
            xr_sb = persist.tile([128, NW, HDA], dt.bfloat16)
            hT_sb = persist.tile([128, H4, NW, 128], dt.bfloat16)
            xr2_sb = persist.tile([128, NW, NC2], dt.bfloat16)
            xl2_stage = persist.tile([128, NW, 128], dt.bfloat16)
            out_stage = persist.tile([128, NW, NCLS], dt.float32)

            # ---- phase 1: xl/al (full graph -> DRAM), xr/ar (own shard -> SBUF) ----
            with (
                tc.tile_pool(name="p1sb", bufs=3) as p1sb,
                tc.tile_pool(name="p1ps", bufs=3, space="PSUM") as p1ps,
                tc.tile_pool(name="p1ps2", bufs=3, space="PSUM") as p1ps2,
            ):
                nc.vector.memset(xl2_stage[:], 0.0)
                nt = 0
                while nt < cfg.n_node_tiles:
                    u = min(8, cfg.n_node_tiles - nt)
                    lt = p1sb.tile([128, FC, 8 * 128], dt.bfloat16, tag="lhsx")
                    nc.sync.dma_start(
                        lt[:, :, :u * 128],
                        xT_d.ap()[:, nt * 128:(nt + u) * 128]
                        .rearrange("(c r) k -> r c k", r=128))
                    ev = p1sb.tile([128, 8, HDA], dt.bfloat16, tag="ev")
                    for ui in range(u):
                        ps = p1ps.tile([128, HD], dt.float32, tag="p1")
                        psa = p1ps2.tile([128, H], dt.float32, tag="p1a")
                        for ci in range(FC):
                            nc.tensor.matmul(ps[:], lt[:, ci, ui * 128:(ui + 1) * 128],
                                             Wl1[:, ci, :],
                                             start=(ci == 0), stop=(ci == FC - 1))
                            nc.tensor.matmul(psa[:], lt[:, ci, ui * 128:(ui + 1) * 128],
                                             WAl[:, ci, :],
                                             start=(ci == 0), stop=(ci == FC - 1))
                        if ui % 2 == 0:
                            nc.scalar.activation(ev[:, ui, :HD], ps[:], AF.Copy)
                            nc.scalar.activation(ev[:, ui, HD:HDA], psa[:], AF.Copy)
                        else:
                            nc.vector.tensor_copy(ev[:, ui, :HD], ps[:])
                            nc.vector.tensor_copy(ev[:, ui, HD:HDA], psa[:])
                    nc.sync.dma_start(
                        xl_dram[nt * 128:(nt + u) * 128, :HDA]
                        .rearrange("(u p) k -> p u k", p=128),
                        ev[:, :u, :])
                    nt += u
                for w0 in range(0, NW, 4):
                    u = min(4, NW - w0)
                    lt = p1sb.tile([128, FC, 8 * 128], dt.bfloat16, tag="lhsx")
                    nc.sync.dma_start(
                        lt[:, :, :u * 128],
                        xTs_d.ap()[:, w0 * 128:(w0 + u) * 128]
                        .rearrange("(c r) k -> r c k", r=128))
                    for ui in range(u):
                        w = w0 + ui
                        ps = p1ps.tile([128, HD], dt.float32, tag="p1")
                        psa = p1ps2.tile([128, H], dt.float32, tag="p1a")
                        for ci in range(FC):
                            nc.tensor.matmul(ps[:], lt[:, ci, ui * 128:(ui + 1) * 128],
                                             Wr1[:, ci, :],
                                             start=(ci == 0), stop=(ci == FC - 1))
                            nc.tensor.matmul(psa[:], lt[:, ci, ui * 128:(ui + 1) * 128],
                                             WAr[:, ci, :],
                                             start=(ci == 0), stop=(ci == FC - 1))
                        # fold both layer-1 biases (and their attention dots) here
                        nc.vector.tensor_add(xr_sb[:, w, :HD], ps[:], xrb[:, :HD])
                        nc.vector.tensor_add(xr_sb[:, w, HD:HDA], psa[:],
                                             xrb[:, HD:HDA])

            # ---- phase 2: layer-1 edges + projections + chunked AllGather ----
            with (
                tc.tile_pool(name="p2g", bufs=4) as p2g,
                tc.tile_pool(name="p2m", bufs=1) as p2m,
                tc.tile_pool(name="p2w", bufs=2) as p2wp,
                tc.tile_pool(name="p2ps", bufs=2, space="PSUM") as p2ps,
                tc.tile_pool(name="p2lg", bufs=1, space="PSUM") as p2lg,
                tc.tile_pool(name="p2acc", bufs=1, space="PSUM") as p2acc,
                tc.tile_pool(name="p2dacc", bufs=1, space="PSUM") as p2dacc,
                tc.tile_pool(name="p2hT", bufs=1, space="PSUM") as p2hT,
                tc.tile_pool(name="p2pj", bufs=1, space="PSUM") as p2pj,
                tc.tile_pool(name="p2s", bufs=2) as p2s,
            ):
                st = {}

                def p2_front(w):
                    Tw = T[w]
                    base = int(toff[w])
                    xlg = p2g.tile([128, Tmax, EL1], dt.bfloat16, tag="xlg")
                    nc.gpsimd.dma_gather(
                        xlg[:, :Tw, :], xl_dram[:],
                        idx16[:, base * 8:(base + Tw) * 8],
                        Tw * 128, Tw * 128, EL1, single_packet=False,
                        queue_num=w % NQ)
                    Mw = p2wp.tile([128, Tmax, 128], dt.bfloat16, tag="Mw")
                    MTw = p2wp.tile([128, Tmax, 128], dt.bfloat16, tag="MTw")
                    nc.sync.dma_start(Mw[:, :Tw, :],
                                      Mmat_d.ap()[base:base + Tw]
                                      .rearrange("t p k -> p t k"))
                    nc.sync.dma_start(MTw[:, :Tw, :],
                                      MTmat_d.ap()[base:base + Tw]
                                      .rearrange("t p k -> p t k"))
                    m_w = p2m.tile([128, Tmax, HD], dt.bfloat16, tag="m")
                    mp_lg = p2lg.tile([128, Tmax * H], dt.float32, tag="lgacc")
                    lgS = p2s.tile([128, Tmax * H], dt.float32, tag="lgS")
                    red = p2s.tile([128, Tmax * H], dt.float32, tag="red")
                    lg = p2s.tile([128, Tmax * H], dt.float32, tag="lg")
                    p_w = p2s.tile([128, Tmax * H], dt.bfloat16, tag="p")
                    h1 = (Tw + 1) // 2
                    for t in range(Tw):
                        mp = p2ps.tile([128, HD], dt.float32, tag="mpre")
                        nc.tensor.matmul(mp[:], MTw[:, t, :], xr_sb[:, w, :HD],
                                         start=True, stop=False)
                        nc.tensor.matmul(mp_lg[:, t * H:(t + 1) * H], MTw[:, t, :],
                                         xr_sb[:, w, HD:HDA], start=True, stop=False)
                        nc.tensor.matmul(mp[:], ident[:], xlg[:, t, :HD],
                                         start=False, stop=True)
                        nc.tensor.matmul(mp_lg[:, t * H:(t + 1) * H], ident[:],
                                         xlg[:, t, HD:HDA], start=False, stop=True)
                        nc.scalar.activation(m_w[:, t, :], mp[:], AF.Relu)
                        if t == h1 - 1 or t == Tw - 1:
                            lo = 0 if t == h1 - 1 else h1
                            hi = t + 1
                            sl = slice(lo * H, hi * H)
                            nc.scalar.activation(lgS[:, sl], mp_lg[:, sl], AF.Copy)
                            nc.vector.tensor_mul(
                                m_w[:, lo:hi, :], m_w[:, lo:hi, :],
                                att1[:].rearrange("p (o k) -> p o k", o=1)
                                    .to_broadcast([128, hi - lo, HD]))
                            nc.vector.tensor_reduce(
                                out=red[:, sl],
                                in_=m_w[:, lo:hi, :]
                                    .rearrange("p t (h d) -> p (t h) d", h=H),
                                op=mybir.AluOpType.add, axis=mybir.AxisListType.X)
                            nc.vector.tensor_add(lg[:, sl], red[:, sl], lgS[:, sl])
                            nc.scalar.activation(p_w[:, sl], lg[:, sl], AF.Exp)
                    # numerator p-muls last: they only feed next window's
                    # scatter, so keep them off the critical DVE chain.
                    # (per-tile 3-D broadcast keeps the DVE in 2x mode)
                    for tt in range(Tw):
                        nc.vector.tensor_mul(
                            xlg[:, tt, :HD]
                                .rearrange("p (h d) -> p h d", h=H),
                            xlg[:, tt, :HD]
                                .rearrange("p (h d) -> p h d", h=H),
                            p_w[:, tt * H:(tt + 1) * H]
                                .rearrange("p (h o) -> p h o", o=1)
                                .to_broadcast([128, H, D]))
                    st[w] = (xlg, Mw, p_w)

                st2 = {}

                def p2_backA(w):
                    Tw = T[w]
                    xlg, Mw, p_w = st.pop(w)
                    out_ps = p2acc.tile([128, HD], dt.float32, tag="oacc")
                    den_ps = p2dacc.tile([128, H], dt.float32, tag="dacc")
                    for t in range(Tw):
                        nc.tensor.matmul(out_ps[:], Mw[:, t, :], xlg[:, t, :HD],
                                         start=(t == 0), stop=(t == Tw - 1))
                        nc.tensor.matmul(den_ps[:], Mw[:, t, :],
                                         p_w[:, t * H:(t + 1) * H],
                                         start=(t == 0), stop=(t == Tw - 1))
                    den = p2s.tile([128, H], dt.float32, tag="den", bufs=1)
                    rec = p2s.tile([128, H], dt.float32, tag="rec", bufs=1)
                    nc.vector.tensor_scalar_max(den[:], den_ps[:], 1e-30)
                    nc.vector.reciprocal(rec[:], den[:])
                    tmp = p2s.tile([128, HD], dt.float32, tag="tmp", bufs=1)
                    nc.vector.tensor_mul(
                        tmp[:].rearrange("p (h d) -> p h d", h=H),
                        out_ps[:].rearrange("p (h d) -> p h d", h=H),
                        rec[:].rearrange("p (h o) -> p h o", o=1)
                            .to_broadcast([128, H, D]))
                    nc.vector.tensor_add(tmp[:], tmp[:], bias1[:])
                    h_w = p2s.tile([128, HD], dt.bfloat16, tag="hw", bufs=2)
                    nc.scalar.activation(h_w[:], tmp[:], AF.Relu)
                    st2[w] = h_w

                def p2_backB(w):
                    h_w = st2.pop(w)
                    hT_ps = p2hT.tile([128, H4, 128], dt.bfloat16, tag="hT")
                    for c4 in range(H4):
                        nc.tensor.transpose(hT_ps[:, c4, :],
                                            h_w[:, c4 * 128:(c4 + 1) * 128], ident[:])
                    nc.scalar.activation(hT_sb[:, :, w, :], hT_ps[:], AF.Copy)
                    pj = p2pj.tile([128, NC2], dt.float32, tag="pj")
                    for c4 in range(H4):
                        nc.tensor.matmul(pj[:], hT_sb[:, c4, w, :], Wl2[:, c4, :],
                                         start=(c4 == 0), stop=(c4 == H4 - 1))
                    nc.scalar.activation(xl2_stage[:, w, :NC2], pj[:], AF.Copy)
                    pj = p2pj.tile([128, NC2], dt.float32, tag="pj")
                    for c4 in range(H4):
                        nc.tensor.matmul(pj[:], hT_sb[:, c4, w, :], Wr2[:, c4, :],
                                         start=(c4 == 0), stop=(c4 == H4 - 1))
                    nc.scalar.activation(xr2_sb[:, w, :], pj[:], AF.Copy)
                    for k, (w0, w1, rows) in enumerate(CHUNKS):
                        if w != w1 - 1:
                            continue
                        s0 = ch_start[k]
                        fw = rows // 128
                        nc.sync.dma_start(
                            xl2_shard[s0:s0 + fw * 128, :]
                            .rearrange("(w p) k -> p w k", p=128),
                            xl2_stage[:, w0:w0 + fw, :])
                        if rows % 128:
                            nc.sync.dma_start(
                                xl2_shard[s0 + fw * 128:s0 + rows, :],
                                xl2_stage[:rows % 128, w0 + fw, :])
                        nc.gpsimd.collective_compute(
                            "AllGather", mybir.AluOpType.bypass,
                            replica_groups=[list(range(cfg.n_cores))],
                            ins=[xl2_shard[s0:s0 + rows]],
                            outs=[xl2_full[cfg.n_cores * s0:
                                           cfg.n_cores * (s0 + rows)]])

                for w in range(NW):
                    p2_front(w)
                    if w >= 1:
                        p2_backA(w - 1)
                    if w >= 2:
                        p2_backB(w - 2)
                p2_backA(NW - 1)
                p2_backB(NW - 2)
                p2_backB(NW - 1)

            # ---- phase 4: layer-2 edge processing ----
            with (
                tc.tile_pool(name="p4g", bufs=5) as p4g,
                tc.tile_pool(name="p4w", bufs=2) as p4wp,
                tc.tile_pool(name="p4c", bufs=1) as p4c,
                tc.tile_pool(name="p4ps", bufs=3, space="PSUM") as p4ps,
                tc.tile_pool(name="p4acc", bufs=2, space="PSUM") as p4acc,
                tc.tile_pool(name="p4s", bufs=2) as p4s,
            ):
                idx16b = p4c.tile([128, TOT * 8], dt.int16, tag="idx2")
                nc.sync.dma_start(idx16b[:], idx2_d.ap())
                st4 = {}

                def p4_front(w):
                    Tw = T[w]
                    base = int(toff[w])
                    x2g = p4g.tile([128, Tmax, 128], dt.bfloat16, tag="x2g")
                    nc.gpsimd.dma_gather(
                        x2g[:, :Tw, :], xl2_full[:],
                        idx16b[:, base * 8:(base + Tw) * 8],
                        Tw * 128, Tw * 128, 128, single_packet=False,
                        queue_num=w % NQ)
                    Mw = p4wp.tile([128, Tmax, 128], dt.bfloat16, tag="Mw2")
                    MTw = p4wp.tile([128, Tmax, 128], dt.bfloat16, tag="MTw2")
                    nc.sync.dma_start(Mw[:, :Tw, :],
                                      Mmat_d.ap()[base:base + Tw]
                                      .rearrange("t p k -> p t k"))
                    nc.sync.dma_start(MTw[:, :Tw, :],
                                      MTmat_d.ap()[base:base + Tw]
                                      .rearrange("t p k -> p t k"))
                    mp = p4ps.tile([128, Tmax, NC2], dt.float32, tag="m2")
                    m2b = p4s.tile([128, Tmax, NCLS], dt.float32, tag="m2b")
                    m2 = p4s.tile([128, Tmax, NCLS], dt.bfloat16, tag="m2l")
                    red2 = p4s.tile([128, Tmax], dt.float32, tag="red2")
                    lg2 = p4s.tile([128, Tmax], dt.float32, tag="lg2")
                    p2w_t = p4s.tile([128, Tmax], dt.bfloat16, tag="p2w")
                    h1 = (Tw + 1) // 2
                    for t in range(Tw):
                        nc.tensor.matmul(mp[:, t, :], MTw[:, t, :],
                                         xr2_sb[:, w, :], start=True, stop=False)
                        nc.tensor.matmul(mp[:, t, :], ident[:],
                                         x2g[:, t, :NC2], start=False, stop=True)
                        if t == h1 - 1 or t == Tw - 1:
                            lo = 0 if t == h1 - 1 else h1
                            hi = t + 1
                            n = hi - lo
                            nc.vector.tensor_add(
                                m2b[:, lo:hi, :], mp[:, lo:hi, :NCLS],
                                b2sum[:, :].rearrange("p (o c) -> p o c", o=1)
                                    .to_broadcast([128, n, NCLS]))
                            nc.scalar.activation(m2[:, lo:hi, :],
                                                 m2b[:, lo:hi, :], AF.Relu)
                            nc.vector.tensor_mul(
                                m2[:, lo:hi, :], m2[:, lo:hi, :],
                                att2[:, :].rearrange("p (o c) -> p o c", o=1)
                                    .to_broadcast([128, n, NCLS]))
                            nc.vector.tensor_reduce(
                                out=red2[:, lo:hi], in_=m2[:, lo:hi, :],
                                op=mybir.AluOpType.add, axis=mybir.AxisListType.X)
                            nc.vector.tensor_add(lg2[:, lo:hi], red2[:, lo:hi],
                                                 mp[:, lo:hi, NCLS])
                            nc.scalar.activation(p2w_t[:, lo:hi], lg2[:, lo:hi],
                                                 AF.Exp, bias=c0[:])
                            nc.vector.tensor_mul(
                                x2g[:, lo:hi, :NCLS],
                                x2g[:, lo:hi, :NCLS],
                                p2w_t[:, lo:hi].rearrange("p (t o) -> p t o", o=1)
                                    .to_broadcast([128, n, NCLS]))
                    st4[w] = (x2g, Mw, p2w_t)

                def p4_back(w):
                    Tw = T[w]
                    x2g, Mw, p2w_t = st4.pop(w)
                    out2_ps = p4acc.tile([128, NCLS], dt.float32, tag="o2acc")
                    den2_ps = p4acc.tile([128, 1], dt.float32, tag="d2acc")
                    for t in range(Tw):
                        nc.tensor.matmul(out2_ps[:], Mw[:, t, :], x2g[:, t, :NCLS],
                                         start=(t == 0), stop=(t == Tw - 1))
                        nc.tensor.matmul(den2_ps[:], Mw[:, t, :], p2w_t[:, t:t + 1],
                                         start=(t == 0), stop=(t == Tw - 1))
                    den2 = p4s.tile([128, 1], dt.float32, tag="den2")
                    rec2 = p4s.tile([128, 1], dt.float32, tag="rec2")
                    nc.vector.tensor_scalar_max(den2[:], den2_ps[:], 1e-30)
                    nc.vector.reciprocal(rec2[:], den2[:])
                    tmp2 = p4s.tile([128, NCLS], dt.float32, tag="tmp2")
                    nc.vector.tensor_mul(
                        tmp2[:].rearrange("p (o c) -> p o c", o=1),
                        out2_ps[:].rearrange("p (o c) -> p o c", o=1),
                        rec2[:].rearrange("p (c o) -> p c o", c=1)
                            .to_broadcast([128, 1, NCLS]))
                    nc.vector.tensor_add(out_stage[:, w, :], tmp2[:], bias2[:])

                for w in range(NW):
                    p4_front(w)
                    if w > 0:
                        p4_back(w - 1)
                p4_back(NW - 1)
                fw, rem = cfg.full_w, cfg.rem
                nc.sync.dma_start(
                    out_d.ap()[:fw * 128, :].rearrange("(w p) k -> p w k", p=128),
                    out_stage[:, :fw, :])
                if rem:
                    nc.sync.dma_start(out_d.ap()[fw * 128:, :],
                                      out_stage[:rem, fw, :])

    nc.compile()
    return nc


_last_result = None


def kernel(**inputs) -> np.ndarray:
    global _last_result
    import os
    cfg = Cfg()
    in_maps, meta = _prep_host(cfg, **inputs)
    nc = build_program(cfg, meta)
    kw = {}
    if os.environ.get("GAT_TRACE"):
        kw = dict(trace=True, tmpdir=os.environ.get("GAT_TRACE_DIR") or None)
    res = run_bass_kernel_spmd(nc, in_maps, core_ids=list(range(cfg.n_cores)), **kw)
    _last_result = res
    out = np.concatenate([res.results[c]["out"] for c in range(cfg.n_cores)], axis=0)
    return out.astype(np.float32)
